# revision 1
# baseline (speedup 1.0000x reference)
"""Trainium2 Bass kernel for the soft-decision-tree ensemble classifier (V3b).

Sharding: 2-way trees x 4-way batch (core c: tree-group c//4, batch-group
c%4). Per-core compute equals the 8-way batch split, but the weight DMA
halves to 2MB and the leaf-dist chain halves. Host sums the two tree-group
partials per batch slice.

All matmuls fp16 over 512-wide virtual tiles (16 tree-tiles x 2 batch
blocks; PSUM matmul outputs are capped at one 512-f32 bank). Batch block v0
runs fully before v1 so its output accumulator drains (copy+DMA) during v1
compute. ACT instructions are batched 8 tiles mid-stream, smaller at the
tail; DMAs are chunked so arrival order matches consumption order and are
issued only from the sync/gpsimd queues. fp8 stage-1 variants (plain and
residual-corrected DoubleRow) were evaluated and rejected: plain fp8 breaks
the 2e-2 error budget (~5%) and the residual-corrected form needs 6
matmuls/tile, which is slower than 4 fp16 matmuls on HW where instruction
time is dominated by the moving stream, not the row count.
"""

import numpy as np

TREE_DEPTH = 6
T, N, D, C = 64, 63, 512, 100
L = 2**TREE_DEPTH
NPAD = 64
TG = 2
BG = 4
TL = T // TG               # 32 trees per core
TNP2 = TL * NPAD           # 2048 node rows
NTIL = TNP2 // 128         # 16 tree tiles
B = 4096
NCORES = 8
BSL = B // BG              # 1024 batch rows per core

_COL_BIAS = 0
_COL_W2 = 16
_CONST_COLS = 32

_GROUPS = [8, 8, 8, 4, 2, 2]   # virtual-tile ACT batches (sum 32)


def _leaf_paths(depth):
    Ll = 2**depth
    idx = np.zeros((Ll, depth), np.int32)
    dr = np.zeros((Ll, depth), np.int32)
    for l in range(Ll):
        node = 0
        for k in range(depth):
            bit = (l >> (depth - 1 - k)) & 1
            idx[l, k] = node
            dr[l, k] = bit
            node = 2 * node + 1 + bit
    return idx, dr


def _pack_amat():
    idx, dr = _leaf_paths(TREE_DEPTH)
    mdir = np.zeros((NPAD, L), np.float32)
    mpath = np.zeros((NPAD, L), np.float32)
    for l in range(L):
        for k in range(TREE_DEPTH):
            n = idx[l, k]
            mpath[n, l] -= 1.0
            if dr[l, k]:
                mdir[n, l] += 1.0
    amat = np.zeros((128, 256), np.float16)
    amat[:NPAD, 0:L] = mdir
    amat[NPAD:, L:128] = mdir
    amat[:NPAD, 128:128 + L] = mpath
    amat[NPAD:, 128 + L:256] = mpath
    return amat


_NC_CACHE = {}


def _build_bass():
    import concourse.bacc as bacc
    import concourse.mybir as mybir
    import concourse.tile as tile
    from concourse.hw_specs import get_activation_tables

    dt = mybir.dt
    f32 = dt.float32
    f32r = dt.float32r
    fp16 = dt.float16
    AF = mybir.ActivationFunctionType
    ALU = mybir.AluOpType
    AX = mybir.AxisListType

    nc = bacc.Bacc("TRN2", target_bir_lowering=False, debug=False,
                   num_devices=NCORES)

    table_id = next(i for i, (_, funcs) in
                    enumerate(get_activation_tables("gen3").items())
                    if AF.Exp in funcs and AF.Ln in funcs)
    nc.scalar.add_instruction(mybir.InstLoadActFuncSet(
        name=f"I-{nc.next_id()}", ins=[], outs=[], act_func_set_id=table_id))

    xt = nc.dram_tensor("xt", [D, BSL], fp16, kind="ExternalInput").ap()
    wt = nc.dram_tensor("wt", [D, TNP2], fp16, kind="ExternalInput").ap()
    consts = nc.dram_tensor("consts", [128, _CONST_COLS], f32r,
                            kind="ExternalInput").ap()
    amat = nc.dram_tensor("amat", [128, 256], fp16, kind="ExternalInput").ap()
    llf = nc.dram_tensor("llf", [TNP2, C], fp16, kind="ExternalInput").ap()
    outs = [nc.dram_tensor(f"out{v}", [C, 512], f32,
                       kind="ExternalOutput").ap() for v in range(2)]

    with tile.TileContext(nc) as tc:
        with (
            tc.tile_pool(name="big", bufs=1) as bigp,
            tc.tile_pool(name="const", bufs=1) as constp,
            tc.tile_pool(name="work", bufs=3) as work,
            tc.tile_pool(name="tmp", bufs=2) as tmpp,
            tc.tile_pool(name="pz", bufs=2, space="PSUM") as pzp,
            tc.tile_pool(name="pp", bufs=2, space="PSUM") as ppp,
            tc.tile_pool(name="po", bufs=1, space="PSUM") as pop,
        ):
            wt_t = [bigp.tile([128, TNP2], fp16, tag=f"wt{j}", name=f"wt{j}")
                    for j in range(4)]
            xt_t = bigp.tile([128, 4 * BSL], fp16, tag="xt")
            consts_t = constp.tile([128, _CONST_COLS], f32r, tag="consts")
            ll_t = bigp.tile([128, NTIL * C], fp16, tag="ll")
            amat_t = constp.tile([128, 256], fp16, tag="amat")

            # ---- DMA plan: tiny consts first, x in 8 pieces, first weight
            # column chunks, leaf logits, remaining weights ----------------
            nc.sync.dma_start(out=consts_t[:], in_=consts[:])
            nc.gpsimd.dma_start(out=amat_t[:], in_=amat[:])
            k = 0

            def dma(out_, in_):
                nonlocal k
                eng = nc.sync if k % 2 == 0 else nc.gpsimd
                eng.dma_start(out=out_, in_=in_)
                k += 1

            HB = BSL // 2
            for j in range(4):
                for h in range(2):
                    dma(xt_t[:, j * BSL + h * HB:j * BSL + (h + 1) * HB],
                        xt[j * 128:(j + 1) * 128, h * HB:(h + 1) * HB])
            # weight chunks: 4 x 128 cols, then 256-col pieces
            for cidx in range(2):
                for j in range(4):
                    cs = cidx * 128
                    dma(wt_t[j][:, cs:cs + 128],
                        wt[j * 128:(j + 1) * 128, cs:cs + 128])
            for h in range(2):
                dma(ll_t[:].rearrange("p (i c) -> p i c", c=C)[
                        :, h * (NTIL // 2):(h + 1) * (NTIL // 2), :],
                    llf.rearrange("(i p) c -> p i c", p=128)[
                        :, h * (NTIL // 2):(h + 1) * (NTIL // 2), :])
            for cidx in range(1, 8):
                for j in range(4):
                    cs = cidx * 256
                    dma(wt_t[j][:, cs:cs + 256],
                        wt[j * 128:(j + 1) * 128, cs:cs + 256])

            adir_ap = amat_t[:, 0:128]
            apath_ap = amat_t[:, 128:256]
            w2_ap = consts_t[:, _COL_W2:_COL_W2 + NTIL].bitcast(f32)

            def bias_ap(i):
                return consts_t[:, _COL_BIAS + i:_COL_BIAS + i + 1].bitcast(f32)

            out_ps = [pop.tile([C, 512], f32, tag=f"outps{v}",
                   name=f"outps{v}") for v in range(2)]

            # ---- leaf-dist chain (lazy issue before first tail) -------
            vt_holder = [None]

            def issue_dist_chain():
                ev_all = bigp.tile([128, NTIL * C], f32, tag="evall")
                nc.scalar.activation(ev_all[:], ll_t[:], AF.Exp)
                sv_all = constp.tile([128, NTIL], f32, tag="svall")
                nc.vector.tensor_reduce(
                    out=sv_all[:],
                    in_=ev_all[:].rearrange("p (i c) -> p i c", c=C),
                    op=ALU.add, axis=AX.X)
                rv_all = constp.tile([128, NTIL], f32, tag="rvall")
                nc.vector.reciprocal(rv_all[:], sv_all[:])
                rw2 = constp.tile([128, NTIL], f32, tag="rw2")
                nc.gpsimd.tensor_tensor(out=rw2[:], in0=rv_all[:],
                                        in1=w2_ap, op=ALU.mult)
                vt_all = bigp.tile([128, NTIL * C], fp16, tag="vtall")
                nc.gpsimd.tensor_tensor(
                    out=vt_all[:].rearrange("p (i c) -> p i c", c=C),
                    in0=ev_all[:].rearrange("p (i c) -> p i c", c=C),
                    in1=rw2[:].unsqueeze(2).broadcast_to([128, NTIL, C]),
                    op=ALU.mult)
                vt_holder[0] = vt_all

            # ---- main pipeline (512-wide virtual tiles, v0 pass first) --
            VB = 512

            def vt_of(u):
                return (u % NTIL, u // NTIL)

            pending_tails = []
            u0 = 0
            for gsz in _GROUPS:
                ta2 = work.tile([128, gsz * VB], fp16, tag=f"ta{gsz}",
                                name=f"ta_{u0}")
                tb2 = work.tile([128, gsz * VB], fp16, tag=f"tb{gsz}",
                                name=f"tb_{u0}")
                for uu in range(gsz):
                    i, v = vt_of(u0 + uu)
                    pz = pzp.tile([128, VB], f32, tag="pz")
                    for j in range(4):
                        nc.tensor.matmul(
                            pz[:],
                            lhsT=wt_t[j][:, i * 128:(i + 1) * 128],
                            rhs=xt_t[:, j * BSL + v * VB:
                                     j * BSL + (v + 1) * VB],
                            start=(j == 0), stop=(j == 3),
                        )
                    nc.vector.tensor_scalar_add(
                        out=ta2[:, uu * VB:(uu + 1) * VB], in0=pz[:],
                        scalar1=bias_ap(i))
                te = tmpp.tile([128, gsz * VB], fp16, tag=f"te{gsz}",
                               name=f"te_{u0}")
                nc.scalar.activation(te[:], ta2[:], AF.Exp)
                nc.scalar.activation(tb2[:], te[:], AF.Ln, bias=1.0,
                                     scale=1.0)

                def group_tail(u0=u0, gsz=gsz, ta2=ta2, tb2=tb2):
                    if vt_holder[0] is None:
                        issue_dist_chain()
                    vt_all = vt_holder[0]
                    h = 0
                    while h < gsz:
                        w_ = min(2, gsz - h)
                        pp = ppp.tile([128, 2 * VB], f32, tag="pp", name="pp")
                        for h2 in range(w_):
                            sl = slice(h2 * VB, (h2 + 1) * VB)
                            nc.tensor.matmul(
                                pp[:, sl], lhsT=adir_ap,
                                rhs=ta2[:, (h + h2) * VB:(h + h2 + 1) * VB],
                                start=True, stop=False)
                            nc.tensor.matmul(
                                pp[:, sl], lhsT=apath_ap,
                                rhs=tb2[:, (h + h2) * VB:(h + h2 + 1) * VB],
                                start=False, stop=True)
                        lp = work.tile([128, 2 * VB], fp16, tag="lp",
                                       name="lp")
                        nc.scalar.activation(lp[:, 0:w_ * VB],
                                             pp[:, 0:w_ * VB], AF.Exp)
                        for h2 in range(w_):
                            i, v = vt_of(u0 + h + h2)
                            nc.tensor.matmul(
                                out_ps[v][:],
                                lhsT=vt_all[:, i * C:(i + 1) * C],
                                rhs=lp[:, h2 * VB:(h2 + 1) * VB],
                                start=(i == 0), stop=(i == NTIL - 1))
                            if i == NTIL - 1:
                                osb = work.tile([C, VB], f32, tag=f"osb{v}",
                                                name=f"osb{v}")
                                nc.vector.tensor_copy(out=osb[:],
                                                      in_=out_ps[v][:])
                                eng = nc.gpsimd if v == 0 else nc.sync
                                eng.dma_start(out=outs[v][:], in_=osb[:])
                        h += w_

                pending_tails.append(group_tail)
                if len(pending_tails) > 1:
                    pending_tails.pop(0)()
                u0 += gsz

            while pending_tails:
                pending_tails.pop(0)()

    nc.finalize()
    return nc


def _get_nc():
    if "nc" not in _NC_CACHE:
        _NC_CACHE["nc"] = _build_bass()
    return _NC_CACHE["nc"]


def _prep_inputs(x, split_weights, split_bias, leaf_logits, tree_weights):
    x = np.asarray(x, np.float32)
    split_weights = np.asarray(split_weights, np.float32)
    split_bias = np.asarray(split_bias, np.float32)
    leaf_logits = np.asarray(leaf_logits, np.float32)
    tree_weights = np.asarray(tree_weights, np.float32)

    w_soft = np.exp(tree_weights - tree_weights.max())
    w_soft = w_soft / w_soft.sum()
    amat = _pack_amat()

    in_maps = []
    for tg in range(TG):
        trees = slice(tg * TL, (tg + 1) * TL)
        wpad = np.zeros((TL, NPAD, D), np.float32)
        wpad[:, :N, :] = split_weights[trees]
        wtT = np.ascontiguousarray(
            wpad.reshape(TNP2, D).T.astype(np.float16))          # [D, TNP2]
        consts = np.zeros((128, _CONST_COLS), np.float32)
        bpad = np.zeros((TL, NPAD), np.float32)
        bpad[:, :N] = split_bias[trees]
        consts[:, _COL_BIAS:_COL_BIAS + NTIL] = bpad.reshape(NTIL, 128).T
        tree_of = (np.arange(NTIL)[None, :] * 2
                   + (np.arange(128)[:, None] // 64)) + tg * TL
        consts[:, _COL_W2:_COL_W2 + NTIL] = 2.0 * w_soft[tree_of]
        llf = np.ascontiguousarray(
            leaf_logits[trees].reshape(TNP2, C).astype(np.float16))
        shared = dict(wt=wtT, consts=consts, llf=llf, amat=amat)
        for bg in range(BG):
            xti = np.ascontiguousarray(
                x[bg * BSL:(bg + 1) * BSL, :].T.astype(np.float16))
            in_maps.append(dict(xt=xti, **shared))
    return in_maps


def kernel(x, split_weights, split_bias, leaf_logits, tree_weights):
    from concourse.bass_utils import run_bass_kernel_spmd

    in_maps = _prep_inputs(x, split_weights, split_bias, leaf_logits,
                           tree_weights)
    nc = _get_nc()
    res = run_bass_kernel_spmd(nc, in_maps, core_ids=list(range(NCORES)))
    out = np.zeros((B, C), np.float32)
    for tg in range(TG):
        for bg in range(BG):
            r = res.results[tg * BG + bg]
            part = np.concatenate([r["out0"], r["out1"]], axis=1).T
            out[bg * BSL:(bg + 1) * BSL] += part
    return np.ascontiguousarray(out)

